# revision 9
# baseline (speedup 1.0000x reference)
"""Chamfer loss kernel for Trainium2 (8 NeuronCores).

Problem: pred/target [4, 3, 8192] channel-first point clouds.
loss = mean_i min_j ||p_i - t_j|| + mean_j min_i ||p_i - t_j||

Strategy:
  d2[i,j] = ||p_i||^2 + ||t_j||^2 - 2 p_i.t_j  is expressed as a single
  K=16 fp16 matmul per tile by augmenting the contraction dim with
  (p2_hi, p2_lo, 1, 1, and hi/lo splits of -2p_d paired with hi/lo of
  t_d).  The hi/lo fp16 split keeps absolute error of d2 at ~1e-6 while
  running the PE at full (1 cycle/column) speed.

  sqrt is monotonic, so mins are taken over d2 and sqrt'd at the end.

  Sharding: core c -> (batch b = c//2, pred-row half h = c%2).
  Each core computes a [4096, 8192] block of d2:
    - row mins (over all 8192 target cols) -> complete for its rows
    - col mins partially (over its 4096 rows), as a [128, 8192]
      per-partition accumulator; partition + cross-core reduction done
      on host (tiny).
  PE produces d2 tiles into PSUM, ScalarE casts fp32->fp16 into SBUF,
  VectorE does the two min-folds in fp16 (2x perf mode).
"""

import numpy as np

B = 4
D = 3
N = 8192
HALF = N // 2  # pred rows per core
NCORES = 8
K = 16  # augmented contraction dim
RT = HALF // 128  # 32 row tiles per core
G = 4  # column groups
GW = N // G  # 2048 cols per group
MMW = 512  # cols per matmul (one PSUM bank)
RW = 512  # rowacc fold width

_CACHE = {}


def _build_nc(passes=1):
    import concourse.bacc as bacc
    import concourse.tile as tile
    from concourse import mybir

    f16 = mybir.dt.float16
    f32 = mybir.dt.float32
    MIN = mybir.AluOpType.min

    nc = bacc.Bacc(
        "TRN2", target_bir_lowering=False, debug=False, num_devices=NCORES
    )
    stat = nc.dram_tensor("stat", [K, HALF], f16, kind="ExternalInput").ap()
    mov = nc.dram_tensor("mov", [K, N], f16, kind="ExternalInput").ap()
    rowmin_o = nc.dram_tensor("rowmin", [128, RT], f32, kind="ExternalOutput").ap()
    colmin_o = nc.dram_tensor("colmin", [128, N], f16, kind="ExternalOutput").ap()

    with tile.TileContext(nc) as tc:
        with (
            tc.tile_pool(name="persist", bufs=1) as persist,
            tc.tile_pool(name="psum", bufs=2, space="PSUM") as psum_pool,
            tc.tile_pool(name="chunk", bufs=3) as chunk_pool,
            tc.tile_pool(name="racc", bufs=2) as racc_pool,
        ):
            stat_sb = persist.tile([K, HALF], f16)
            mov_sb = persist.tile([K, N], f16)
            colacc = persist.tile([128, N], f16)
            rowmins = persist.tile([128, RT], f32)
            nc.sync.dma_start(stat_sb[:], stat)
            nc.sync.dma_start(mov_sb[:], mov)

            for r in range(RT * passes):
                r = r % RT
                lhsT = stat_sb[:, r * 128 : (r + 1) * 128]
                rowacc = racc_pool.tile([128, RW], f16)
                for g in range(G):
                    pt = psum_pool.tile([128, GW], f32)
                    for s in range(GW // MMW):
                        c0 = g * GW + s * MMW
                        nc.tensor.matmul(
                            pt[:, s * MMW : (s + 1) * MMW],
                            lhsT,
                            mov_sb[:, c0 : c0 + MMW],
                            start=True,
                            stop=True,
                        )
                    ck = chunk_pool.tile([128, GW], f16)
                    nc.scalar.copy(ck[:], pt[:])
                    csl = colacc[:, g * GW : (g + 1) * GW]
                    if r == 0:
                        nc.vector.tensor_copy(csl, ck[:])
                    else:
                        nc.vector.tensor_tensor(csl, ck[:], csl, MIN)
                    for s in range(GW // RW):
                        cks = ck[:, s * RW : (s + 1) * RW]
                        if g == 0 and s == 0:
                            nc.vector.tensor_copy(rowacc[:], cks)
                        else:
                            nc.vector.tensor_tensor(rowacc[:], cks, rowacc[:], MIN)
                nc.vector.tensor_reduce(
                    rowmins[:, r : r + 1],
                    rowacc[:],
                    mybir.AxisListType.X,
                    MIN,
                )
            nc.sync.dma_start(rowmin_o, rowmins[:])
            nc.sync.dma_start(colmin_o, colacc[:])
    nc.compile()
    return nc


def _get_nc():
    if "nc" not in _CACHE:
        _CACHE["nc"] = _build_nc()
    return _CACHE["nc"]


def _split16(x):
    hi = x.astype(np.float16)
    lo = (x - hi.astype(np.float32)).astype(np.float16)
    return hi, lo


def _prep_batch(p, t):
    """p, t: [3, N] fp32 -> (S [K, N] fp16 stationary, M [K, N] fp16 moving)
    with d2[i, j] = sum_k S[k, i] * M[k, j] to ~1e-6 absolute."""
    p2 = (p * p).sum(axis=0)
    t2 = (t * t).sum(axis=0)
    S = np.empty((K, N), np.float16)
    M = np.empty((K, N), np.float16)
    S[0], S[1] = _split16(p2)
    M[0] = 1.0
    M[1] = 1.0
    S[2] = 1.0
    S[3] = 1.0
    M[2], M[3] = _split16(t2)
    for d in range(D):
        ah, al = _split16(-2.0 * p[d])
        bh, bl = _split16(t[d])
        base = 4 + 4 * d
        S[base + 0] = ah
        M[base + 0] = bh
        S[base + 1] = ah
        M[base + 1] = bl
        S[base + 2] = al
        M[base + 2] = bh
        S[base + 3] = al
        M[base + 3] = bl
    return S, M


def _make_in_maps(pred, target):
    pred = np.asarray(pred, dtype=np.float32)
    target = np.asarray(target, dtype=np.float32)
    in_maps = []
    for c in range(NCORES):
        b, h = divmod(c, 2)
        S, M = _prep_batch(pred[b], target[b])
        in_maps.append(
            {
                "stat": np.ascontiguousarray(S[:, h * HALF : (h + 1) * HALF]),
                "mov": M,
            }
        )
    return in_maps


def _finish(results):
    """results: list of 8 dicts with 'rowmin' [128, RT] f32, 'colmin' [128, N] f16."""
    row_total = 0.0
    col_total = 0.0
    for b in range(B):
        colparts = []
        for h in range(2):
            out = results[2 * b + h]
            rm = np.asarray(out["rowmin"], dtype=np.float32)  # [128, RT]
            # row index within half = r*128 + p -> transpose to [RT, 128]
            rd2 = np.maximum(rm.T.reshape(-1), 0.0)
            row_total += np.sqrt(rd2, dtype=np.float64).sum()
            colparts.append(
                np.asarray(out["colmin"], dtype=np.float32).min(axis=0)
            )
        cd2 = np.maximum(np.minimum(colparts[0], colparts[1]), 0.0)
        col_total += np.sqrt(cd2, dtype=np.float64).sum()
    loss = row_total / (B * N) + col_total / (B * N)
    return np.array(loss, dtype=np.float32)


def _run(in_maps, trace=False, nc=None):
    from concourse.bass_utils import run_bass_kernel_spmd

    if nc is None:
        nc = _get_nc()
    res = run_bass_kernel_spmd(
        nc, in_maps, list(range(NCORES)), trace=trace
    )
    return res


def kernel(pred, target):
    res = _run(_make_in_maps(pred, target), trace=False)
    return _finish(res.results)


# revision 13
# speedup vs baseline: 23.1028x; 23.1028x over previous
"""Chamfer loss kernel for Trainium2 (8 NeuronCores).

Problem: pred/target [4, 3, 8192] channel-first point clouds.
loss = mean_i min_j ||p_i - t_j|| + mean_j min_i ||p_i - t_j||

Strategy:
  d2[i,j] = ||p_i||^2 + ||t_j||^2 - 2 p_i.t_j  is expressed as a single
  K=16 fp16 matmul per tile by augmenting the contraction dim with
  (p2_hi, p2_lo, 1, 1, and hi/lo splits of -2p_d paired with hi/lo of
  t_d).  The hi/lo fp16 split keeps absolute error of d2 at ~1e-6 while
  running the PE at full (1 cycle/column) speed.

  sqrt is monotonic, so mins are taken over d2 and sqrt'd at the end.

  Sharding: core c -> (batch b = c//2, pred-row half h = c%2).
  Each core computes a [4096, 8192] block of d2:
    - row mins (over all 8192 target cols) -> complete for its rows
    - col mins partially (over its 4096 rows), as a [128, 8192]
      per-partition accumulator; partition + cross-core reduction done
      on host (tiny).
  PE produces d2 tiles into PSUM, ScalarE casts fp32->fp16 into SBUF,
  VectorE does the two min-folds in fp16 (2x perf mode).
"""

import numpy as np

B = 4
D = 3
N = 8192
HALF = N // 2  # pred rows per core
NCORES = 8
K = 16  # augmented contraction dim
RT = HALF // 128  # 32 row tiles per core
G = 4  # column groups
GW = N // G  # 2048 cols per group
MMW = 512  # cols per matmul (one PSUM bank)
RW = 512  # rowacc fold width

_CACHE = {}


def _build_nc(passes=1, loop_n=None):
    """passes: statically unroll the body N times (grows the NEFF).
    loop_n: wrap the body in a device-side For_i loop executed loop_n
    times — constant program size, used for timing (delta between two
    loop_n values isolates pure HW execution time)."""
    import concourse.bacc as bacc
    import concourse.tile as tile
    from concourse import mybir

    f16 = mybir.dt.float16
    f32 = mybir.dt.float32
    MIN = mybir.AluOpType.min

    nc = bacc.Bacc(
        "TRN2", target_bir_lowering=False, debug=False, num_devices=NCORES
    )
    stat = nc.dram_tensor("stat", [K, HALF], f16, kind="ExternalInput").ap()
    mov = nc.dram_tensor("mov", [K, N], f16, kind="ExternalInput").ap()
    rowmin_o = nc.dram_tensor("rowmin", [128, RT], f32, kind="ExternalOutput").ap()
    colmin_o = nc.dram_tensor("colmin", [128, N], f16, kind="ExternalOutput").ap()

    with tile.TileContext(nc) as tc:
        with (
            tc.tile_pool(name="persist", bufs=1) as persist,
            tc.tile_pool(name="psum", bufs=2, space="PSUM") as psum_pool,
            tc.tile_pool(name="chunk", bufs=3) as chunk_pool,
            tc.tile_pool(name="racc", bufs=2) as racc_pool,
        ):
            stat_sb = persist.tile([K, HALF], f16)
            mov_sb = persist.tile([K, N], f16)
            colacc = persist.tile([128, N], f16)
            rowmins = persist.tile([128, RT], f32)
            nc.sync.dma_start(stat_sb[:], stat)
            nc.sync.dma_start(mov_sb[:], mov)

            import contextlib

            loop_cm = (
                tc.For_i(0, loop_n, 1)
                if loop_n is not None
                else contextlib.nullcontext()
            )
            with loop_cm:
                _body(nc, tc, mybir, stat_sb, mov_sb, colacc, rowmins,
                      psum_pool, chunk_pool, racc_pool, passes)
            nc.sync.dma_start(rowmin_o, rowmins[:])
            nc.sync.dma_start(colmin_o, colacc[:])
    nc.compile()
    return nc


def _body(nc, tc, mybir, stat_sb, mov_sb, colacc, rowmins,
          psum_pool, chunk_pool, racc_pool, passes):
    f16 = mybir.dt.float16
    f32 = mybir.dt.float32
    MIN = mybir.AluOpType.min
    for r in range(RT * passes):
                r = r % RT
                lhsT = stat_sb[:, r * 128 : (r + 1) * 128]
                rowacc = racc_pool.tile([128, RW], f16)
                for g in range(G):
                    pt = psum_pool.tile([128, GW], f32)
                    for s in range(GW // MMW):
                        c0 = g * GW + s * MMW
                        nc.tensor.matmul(
                            pt[:, s * MMW : (s + 1) * MMW],
                            lhsT,
                            mov_sb[:, c0 : c0 + MMW],
                            start=True,
                            stop=True,
                        )
                    ck = chunk_pool.tile([128, GW], f16)
                    nc.scalar.copy(ck[:], pt[:])
                    csl = colacc[:, g * GW : (g + 1) * GW]
                    if r == 0:
                        nc.vector.tensor_copy(csl, ck[:])
                    else:
                        nc.vector.tensor_tensor(csl, ck[:], csl, MIN)
                    for s in range(GW // RW):
                        cks = ck[:, s * RW : (s + 1) * RW]
                        if g == 0 and s == 0:
                            nc.vector.tensor_copy(rowacc[:], cks)
                        else:
                            nc.vector.tensor_tensor(rowacc[:], cks, rowacc[:], MIN)
                nc.vector.tensor_reduce(
                    rowmins[:, r : r + 1],
                    rowacc[:],
                    mybir.AxisListType.X,
                    MIN,
                )


def _get_nc():
    if "nc" not in _CACHE:
        _CACHE["nc"] = _build_nc()
    return _CACHE["nc"]


def _split16(x):
    hi = x.astype(np.float16)
    lo = (x - hi.astype(np.float32)).astype(np.float16)
    return hi, lo


def _prep_batch(p, t):
    """p, t: [3, N] fp32 -> (S [K, N] fp16 stationary, M [K, N] fp16 moving)
    with d2[i, j] = sum_k S[k, i] * M[k, j] to ~1e-6 absolute."""
    p2 = (p * p).sum(axis=0)
    t2 = (t * t).sum(axis=0)
    S = np.empty((K, N), np.float16)
    M = np.empty((K, N), np.float16)
    S[0], S[1] = _split16(p2)
    M[0] = 1.0
    M[1] = 1.0
    S[2] = 1.0
    S[3] = 1.0
    M[2], M[3] = _split16(t2)
    for d in range(D):
        ah, al = _split16(-2.0 * p[d])
        bh, bl = _split16(t[d])
        base = 4 + 4 * d
        S[base + 0] = ah
        M[base + 0] = bh
        S[base + 1] = ah
        M[base + 1] = bl
        S[base + 2] = al
        M[base + 2] = bh
        S[base + 3] = al
        M[base + 3] = bl
    return S, M


def _make_in_maps(pred, target):
    pred = np.asarray(pred, dtype=np.float32)
    target = np.asarray(target, dtype=np.float32)
    in_maps = []
    for c in range(NCORES):
        b, h = divmod(c, 2)
        S, M = _prep_batch(pred[b], target[b])
        in_maps.append(
            {
                "stat": np.ascontiguousarray(S[:, h * HALF : (h + 1) * HALF]),
                "mov": M,
            }
        )
    return in_maps


def _finish(results):
    """results: list of 8 dicts with 'rowmin' [128, RT] f32, 'colmin' [128, N] f16."""
    row_total = 0.0
    col_total = 0.0
    for b in range(B):
        colparts = []
        for h in range(2):
            out = results[2 * b + h]
            rm = np.asarray(out["rowmin"], dtype=np.float32)  # [128, RT]
            # row index within half = r*128 + p -> transpose to [RT, 128]
            rd2 = np.maximum(rm.T.reshape(-1), 0.0)
            row_total += np.sqrt(rd2, dtype=np.float64).sum()
            colparts.append(
                np.asarray(out["colmin"], dtype=np.float32).min(axis=0)
            )
        cd2 = np.maximum(np.minimum(colparts[0], colparts[1]), 0.0)
        col_total += np.sqrt(cd2, dtype=np.float64).sum()
    loss = row_total / (B * N) + col_total / (B * N)
    return np.array(loss, dtype=np.float32)


def _run(in_maps, trace=False, nc=None):
    from concourse.bass_utils import run_bass_kernel_spmd

    if nc is None:
        nc = _get_nc()
    res = run_bass_kernel_spmd(
        nc, in_maps, list(range(NCORES)), trace=trace
    )
    return res


def kernel(pred, target):
    res = _run(_make_in_maps(pred, target), trace=False)
    return _finish(res.results)


# revision 27
# speedup vs baseline: 161.9264x; 7.0089x over previous
"""Chamfer loss kernel for Trainium2 (8 NeuronCores).

Problem: pred/target [4, 3, 8192] channel-first point clouds.
loss = mean_i min_j ||p_i - t_j|| + mean_j min_i ||p_i - t_j||

Strategy:
  d2[i,j] = ||p_i||^2 + ||t_j||^2 - 2 p_i.t_j  is expressed as a single
  K=16 fp16 matmul per tile by augmenting the contraction dim with
  (p2_hi, p2_lo, 1, 1, and hi/lo splits of -2p_d paired with hi/lo of
  t_d).  The hi/lo fp16 split keeps absolute error of d2 at ~1e-6 while
  running the PE at full (1 cycle/column) speed.

  sqrt is monotonic, so mins are taken over d2 and sqrt'd at the end.

  Sharding: core c -> (batch b = c//2, pred-row half h = c%2).
  Each core computes a [4096, 8192] block of d2:
    - row mins (over all 8192 target cols) -> complete for its rows
    - col mins partially (over its 4096 rows), as a [128, 8192]
      per-partition accumulator; partition + cross-core reduction done
      on host (tiny).
  PE produces d2 tiles into PSUM, ScalarE casts fp32->fp16 into SBUF,
  VectorE does the two min-folds in fp16 (2x perf mode).
"""

import numpy as np

B = 4
D = 3
N = 8192
HALF = N // 2  # pred rows per core
NCORES = 8
K = 16  # augmented contraction dim
RT = HALF // 128  # 32 row tiles per core
G = 4  # column groups
GW = N // G  # 2048 cols per group
PW = 1024  # cols per PSUM tile (2 banks; bufs=3 -> 6 banks, finale gets 2)
MMW = 512  # cols per matmul (one PSUM bank)
NT = N // 128  # 64 transpose blocks in the colmin finale

_CACHE = {}


def _build_nc(passes=1, loop_n=None, stage="full"):
    """passes: statically unroll the body N times (grows the NEFF).
    loop_n: wrap the body in a device-side For_i loop executed loop_n
    times — constant program size, used for timing (delta between two
    loop_n values isolates pure HW execution time)."""
    import concourse.bacc as bacc
    import concourse.tile as tile
    from concourse import mybir

    f16 = mybir.dt.float16
    f32 = mybir.dt.float32
    MIN = mybir.AluOpType.min

    nc = bacc.Bacc(
        "TRN2", target_bir_lowering=False, debug=False, num_devices=NCORES
    )
    stat = nc.dram_tensor("stat", [K, HALF], f16, kind="ExternalInput").ap()
    mov = nc.dram_tensor("mov", [K, N], f16, kind="ExternalInput").ap()
    ident = nc.dram_tensor("ident", [128, 128], f32, kind="ExternalInput").ap()
    rowmin_o = nc.dram_tensor("rowmin", [128, RT], f32, kind="ExternalOutput").ap()
    colmin_o = nc.dram_tensor("colmin", [128, NT], f32, kind="ExternalOutput").ap()

    with tile.TileContext(nc) as tc:
        with (
            tc.tile_pool(name="persist", bufs=1) as persist,
            tc.tile_pool(name="psum", bufs=3, space="PSUM") as psum_pool,
            tc.tile_pool(name="psumf", bufs=2, space="PSUM") as psumf_pool,
            tc.tile_pool(name="chunk", bufs=3) as chunk_pool,
            tc.tile_pool(name="racc", bufs=2) as racc_pool,
        ):
            stat_sb = persist.tile([K, HALF], f16)
            mov_sb = persist.tile([K, N], f16)
            ident_sb = persist.tile([128, 128], f32)
            colacc = persist.tile([128, N], f16)
            colacc32 = persist.tile([128, N], f32)
            rowmins = persist.tile([128, RT], f32)
            colmins = persist.tile([128, NT], f32)
            nc.sync.dma_start(stat_sb[:], stat)
            nc.sync.dma_start(mov_sb[:], mov)
            nc.sync.dma_start(ident_sb[:], ident)
            if stage != "full":
                nc.vector.memset(colacc[:], 0.0)
                nc.vector.memset(rowmins[:], 0.0)

            import contextlib

            loop_cm = (
                tc.For_i(0, loop_n, 1)
                if loop_n is not None
                else contextlib.nullcontext()
            )
            with loop_cm:
                _body(nc, tc, mybir, stat_sb, mov_sb, colacc, rowmins,
                      psum_pool, chunk_pool, racc_pool, passes, stage)
                # --- colmin finale: cross-partition reduce of colacc ---
                # cast fp16 -> fp32 (ACT), transpose 128x128 blocks on PE,
                # packed min-reduce 4 blocks per PSUM tile on DVE.
                MIN = mybir.AluOpType.min
                for q in range(G):
                    nc.scalar.copy(
                        colacc32[:, q * GW : (q + 1) * GW],
                        colacc[:, q * GW : (q + 1) * GW],
                    )
                for j in range(NT // 4):
                    pf = psumf_pool.tile([128, 512], f32)
                    for kk in range(4):
                        t = 4 * j + kk
                        nc.tensor.matmul(
                            pf[:, kk * 128 : (kk + 1) * 128],
                            colacc32[:, t * 128 : (t + 1) * 128],
                            ident_sb[:],
                            is_transpose=True,
                            start=True,
                            stop=True,
                        )
                    nc.vector.tensor_reduce(
                        colmins[:, 4 * j : 4 * j + 4],
                        pf[:].rearrange("p (b f) -> p b f", b=4),
                        mybir.AxisListType.X,
                        MIN,
                    )
            nc.sync.dma_start(rowmin_o, rowmins[:])
            nc.sync.dma_start(colmin_o, colmins[:])
    nc.compile()
    return nc


def _body(nc, tc, mybir, stat_sb, mov_sb, colacc, rowmins,
          psum_pool, chunk_pool, racc_pool, passes, stage="full"):
    f16 = mybir.dt.float16
    f32 = mybir.dt.float32
    MIN = mybir.AluOpType.min
    if stage.startswith("dve"):
        # pure-DVE microbench: 256 tensor_tensor min ops on [128, GW] SBUF
        dt = f16 if stage == "dvef16" else f32
        a = chunk_pool.tile([128, GW], dt, tag="mba")
        b = chunk_pool.tile([128, GW], dt, tag="mbb")
        nc.vector.memset(a[:], 1.0)
        nc.vector.memset(b[:], 2.0)
        for _ in range(256):
            nc.vector.tensor_tensor(b[:], a[:], b[:], MIN)
        return
    if stage == "actonly":
        # pure-ACT microbench: 128 PSUM->SBUF cast copies [128, PW]
        pt = psum_pool.tile([128, PW], f32, tag="mbp")
        ck = chunk_pool.tile([128, PW], f16, tag="mbc")
        nc.vector.memset(pt[:], 3.0)
        for _ in range(128):
            nc.scalar.copy(ck[:], pt[:])
        return
    do_act = stage in ("mm_act", "mm_act_col", "mm_act_row", "full")
    do_col = stage in ("mm_act_col", "full")
    do_row = stage in ("mm_act_row", "full")
    HW_ = GW // 2  # 1024
    for r in range(RT * passes):
        r = r % RT
        lhsT = stat_sb[:, r * 128 : (r + 1) * 128]
        rowacc = racc_pool.tile([128, HW_], f16)
        for g in range(G):
            ck = chunk_pool.tile([128, GW], f16)
            for h in range(GW // PW):
                pt = psum_pool.tile([128, PW], f32)
                for s in range(PW // MMW):
                    c0 = g * GW + h * PW + s * MMW
                    nc.tensor.matmul(
                        pt[:, s * MMW : (s + 1) * MMW],
                        lhsT,
                        mov_sb[:, c0 : c0 + MMW],
                        start=True,
                        stop=True,
                    )
                if do_act:
                    nc.scalar.copy(ck[:, h * PW : (h + 1) * PW], pt[:])
            if do_col:
                csl = colacc[:, g * GW : (g + 1) * GW]
                if r == 0:
                    nc.vector.tensor_copy(csl, ck[:])
                else:
                    nc.vector.tensor_tensor(csl, ck[:], csl, MIN)
            if do_row:
                if g == 0:
                    # initializes rowacc = min(ck_left, ck_right)
                    nc.vector.tensor_tensor(
                        rowacc[:], ck[:, :HW_], ck[:, HW_:], MIN
                    )
                else:
                    ht = chunk_pool.tile([128, HW_], f16, tag="ht")
                    nc.vector.tensor_tensor(
                        ht[:], ck[:, :HW_], ck[:, HW_:], MIN
                    )
                    nc.vector.tensor_tensor(rowacc[:], ht[:], rowacc[:], MIN)
        if do_row:
            rfin = chunk_pool.tile([128, HW_ // 2], f16, tag="rfin")
            nc.vector.tensor_tensor(
                rfin[:], rowacc[:, : HW_ // 2], rowacc[:, HW_ // 2 :], MIN
            )
            nc.vector.tensor_reduce(
                rowmins[:, r : r + 1],
                rfin[:],
                mybir.AxisListType.X,
                MIN,
            )


def _get_nc():
    if "nc" not in _CACHE:
        _CACHE["nc"] = _build_nc()
    return _CACHE["nc"]


def _split16(x):
    hi = x.astype(np.float16)
    lo = (x - hi.astype(np.float32)).astype(np.float16)
    return hi, lo


def _prep_batch(p, t):
    """p, t: [3, N] fp32 -> (S [K, N] fp16 stationary, M [K, N] fp16 moving)
    with d2[i, j] = sum_k S[k, i] * M[k, j] to ~1e-6 absolute."""
    p2 = (p * p).sum(axis=0)
    t2 = (t * t).sum(axis=0)
    S = np.empty((K, N), np.float16)
    M = np.empty((K, N), np.float16)
    S[0], S[1] = _split16(p2)
    M[0] = 1.0
    M[1] = 1.0
    S[2] = 1.0
    S[3] = 1.0
    M[2], M[3] = _split16(t2)
    for d in range(D):
        ah, al = _split16(-2.0 * p[d])
        bh, bl = _split16(t[d])
        base = 4 + 4 * d
        S[base + 0] = ah
        M[base + 0] = bh
        S[base + 1] = ah
        M[base + 1] = bl
        S[base + 2] = al
        M[base + 2] = bh
        S[base + 3] = al
        M[base + 3] = bl
    return S, M


def _make_in_maps(pred, target):
    pred = np.asarray(pred, dtype=np.float32)
    target = np.asarray(target, dtype=np.float32)
    ident = np.eye(128, dtype=np.float32)
    in_maps = []
    for c in range(NCORES):
        b, h = divmod(c, 2)
        S, M = _prep_batch(pred[b], target[b])
        in_maps.append(
            {
                "stat": np.ascontiguousarray(S[:, h * HALF : (h + 1) * HALF]),
                "mov": M,
                "ident": ident,
            }
        )
    return in_maps


def _finish(results):
    """results: list of 8 dicts with 'rowmin' [128, RT] f32 and
    'colmin' [128, NT] f32 (colmin[p, t] = min_i d2[i, 128*t + p])."""
    row_total = 0.0
    col_total = 0.0
    for b in range(B):
        colparts = []
        for h in range(2):
            out = results[2 * b + h]
            rm = np.asarray(out["rowmin"], dtype=np.float32)  # [128, RT]
            # row index within half = r*128 + p -> transpose to [RT, 128]
            rd2 = np.maximum(rm.T.reshape(-1), 0.0)
            row_total += np.sqrt(rd2, dtype=np.float64).sum()
            # column j = 128*t + p -> transpose [NT, 128] then flatten
            colparts.append(
                np.asarray(out["colmin"], dtype=np.float32).T.reshape(-1)
            )
        cd2 = np.maximum(np.minimum(colparts[0], colparts[1]), 0.0)
        col_total += np.sqrt(cd2, dtype=np.float64).sum()
    loss = row_total / (B * N) + col_total / (B * N)
    return np.array(loss, dtype=np.float32)


def _run(in_maps, trace=False, nc=None):
    from concourse.bass_utils import run_bass_kernel_spmd

    if nc is None:
        nc = _get_nc()
    res = run_bass_kernel_spmd(
        nc, in_maps, list(range(NCORES)), trace=trace
    )
    return res


def kernel(pred, target):
    res = _run(_make_in_maps(pred, target), trace=False)
    return _finish(res.results)
